# revision 1
# baseline (speedup 1.0000x reference)
"""Transformer block (pre-LN attention + FFN) on 8 TRN2 NeuronCores.

Sharding: batch x head tensor-parallel for attention, sequence-parallel for
LN/FFN/residual. Core c (b = c//4, j = c%4):
  - owns token shard [512j : 512j+512) of batch b for LN1/LN2/FFN/residual
  - owns heads [4j : 4j+4) of batch b for attention (all 2048 tokens)
Collectives (per-batch groups [[0..3],[4..7]]):
  - AllGather of transposed LN1 output hT (bf16) so every core sees all tokens
  - ReduceScatter (add) of the attention projection partial sums back to
    token shards.
All programs are identical across cores (SPMD); only input data differs.

Matmul dtypes: float32r (full-rate fp32, no cast needed) for the
weight-side matmuls fed by f32 DRAM (FFN1), bf16 for attention internals +
QKV/proj/FFN2 (operands produced on-chip, cast free on evacuation).
"""

import numpy as np

import concourse.bass as bass
import concourse.mybir as mybir
import concourse.tile as tile
from concourse import bacc
from concourse.bass_utils import run_bass_kernel_spmd
from concourse.masks import make_identity

P = 128
C = 1024          # n_embd
KT = C // P       # 8 c-tiles
T_OWN = 512       # tokens per core (sequence shard)
IT = T_OWN // P   # 4 own t-tiles
T_ALL = 2048      # tokens per batch
NH = 4            # heads per core
D = 64            # head dim
DL = NH * D       # 256 local head features
FF = 4096
FMT = FF // P     # 32 ffn m-tiles
CH = 256          # attention chunk
QC = T_ALL // CH  # 8 chunks
EPS = 1e-5
SCALE = 1.0 / 32.0  # C ** -0.5
GROUPS = [[0, 1, 2, 3], [4, 5, 6, 7]]
NCORES = 8

f32 = mybir.dt.float32
f32r = mybir.dt.float32r
bf16 = mybir.dt.bfloat16
AX = mybir.AxisListType
ALU = mybir.AluOpType
ACT_F = mybir.ActivationFunctionType


def _layer_norm(nc, sb, st, x_view, w_rep, b_rep, out_view, tmp_tag):
    """Token-major LN over free axis. x_view/out_view: [P, IT, C].
    Stats vectorized across the IT tiles; sum-of-squares via ACT Square with
    fused row-accumulate."""
    ssum = sb.tile([P, IT], f32, tag=tmp_tag + "s", name=f"ln_s_{tmp_tag}")
    sqs = sb.tile([P, IT], f32, tag=tmp_tag + "q", name=f"ln_q_{tmp_tag}")
    for i in range(IT):
        nc.vector.tensor_reduce(ssum[:, i:i + 1], x_view[:, i, :], AX.X, ALU.add)
        sq = st.tile([P, C], f32, tag="outev", bufs=2, name=f"ln_sq_{tmp_tag}_{i}")
        nc.scalar.activation(sq[:], x_view[:, i, :], ACT_F.Square,
                             accum_out=sqs[:, i:i + 1])
    mu = sb.tile([P, IT], f32, tag=tmp_tag + "mu", name=f"ln_mu_{tmp_tag}")
    nc.vector.tensor_scalar_mul(mu[:], ssum[:], 1.0 / C)
    var = sb.tile([P, IT], f32, tag=tmp_tag + "v", name=f"ln_v_{tmp_tag}")
    nc.vector.tensor_scalar_mul(var[:], sqs[:], 1.0 / C)
    musq = sb.tile([P, IT], f32, tag=tmp_tag + "m2", name=f"ln_m2_{tmp_tag}")
    nc.vector.tensor_mul(out=musq[:], in0=mu[:], in1=mu[:])
    nc.vector.tensor_sub(out=var[:], in0=var[:], in1=musq[:])
    nc.vector.tensor_scalar_add(var[:], var[:], EPS)
    rv = sb.tile([P, IT], f32, tag=tmp_tag + "rv", name=f"ln_rv_{tmp_tag}")
    nc.vector.reciprocal(rv[:], var[:])
    rstd = sb.tile([P, IT], f32, tag=tmp_tag + "rs", name=f"ln_rs_{tmp_tag}")
    nc.scalar.sqrt(rstd[:], rv[:])
    for i in range(IT):
        tmp = st.tile([P, C], f32, tag="outev", bufs=2,
                      name=f"ln_tmp_{tmp_tag}_{i}")
        nc.vector.tensor_scalar(
            out=tmp[:], in0=x_view[:, i, :], scalar1=mu[:, i:i + 1],
            scalar2=rstd[:, i:i + 1], op0=ALU.subtract, op1=ALU.mult)
        nc.vector.tensor_mul(out=tmp[:], in0=tmp[:], in1=w_rep[:])
        nc.vector.tensor_tensor(out=out_view[:, i, :], in0=tmp[:], in1=b_rep[:],
                                op=ALU.add)


def build(stage=9, debug=False):
    nc = bacc.Bacc("TRN2", target_bir_lowering=False, debug=False,
                   num_devices=NCORES)
    _build_graph(nc, stage, debug)
    nc.compile()
    return nc


def _build_graph(nc, stage, debug=False):

    x_ext = nc.dram_tensor("x", [T_OWN, C], f32, kind="ExternalInput").ap()
    wq_ext = nc.dram_tensor("wq", [C, DL], f32, kind="ExternalInput").ap()
    wk_ext = nc.dram_tensor("wk", [C, DL], f32, kind="ExternalInput").ap()
    wv_ext = nc.dram_tensor("wv", [C, DL], f32, kind="ExternalInput").ap()
    wp_ext = nc.dram_tensor("wp", [DL, C], f32, kind="ExternalInput").ap()
    w1_ext = nc.dram_tensor("w1", [C, FF], f32r, kind="ExternalInput").ap()
    w2_ext = nc.dram_tensor("w2", [FF, C], f32, kind="ExternalInput").ap()
    bproj_ext = nc.dram_tensor("bproj", [C], f32, kind="ExternalInput").ap()
    b1_ext = nc.dram_tensor("b1", [FF], f32, kind="ExternalInput").ap()
    b2_ext = nc.dram_tensor("b2", [C], f32, kind="ExternalInput").ap()
    ln1w_ext = nc.dram_tensor("ln1w", [C], f32, kind="ExternalInput").ap()
    ln1b_ext = nc.dram_tensor("ln1b", [C], f32, kind="ExternalInput").ap()
    ln2w_ext = nc.dram_tensor("ln2w", [C], f32, kind="ExternalInput").ap()
    ln2b_ext = nc.dram_tensor("ln2b", [C], f32, kind="ExternalInput").ap()
    out_ext = nc.dram_tensor("out", [T_OWN, C], f32, kind="ExternalOutput").ap()
    dbg = {}
    if debug:
        dbg["mask"] = nc.dram_tensor("dbg_mask", [P, 4, CH], f32,
                                     kind="ExternalOutput").ap()
        dbg["ex"] = nc.dram_tensor("dbg_ex", [P, 2, 2 * CH], f32,
                                   kind="ExternalOutput").ap()
        dbg["aps"] = nc.dram_tensor("dbg_aps", [P, 2 * (D + 1)], f32,
                                    kind="ExternalOutput").ap()

    with tile.TileContext(nc) as tc:
        with (
            tc.tile_pool(name="sb", bufs=1) as sb,
            tc.tile_pool(name="st", bufs=3) as st,    # streaming stages
            tc.tile_pool(name="ps", bufs=1, space="PSUM") as ps,
            tc.tile_pool(name="dram", bufs=1, space="DRAM") as dram,
        ):
            # ---- constants / replicated vectors ----
            id_bf = sb.tile([P, P], bf16)
            make_identity(nc, id_bf[:])
            id_f32 = sb.tile([P, P], f32)
            make_identity(nc, id_f32[:])
            id_fr = sb.tile([P, P], f32r)
            nc.vector.tensor_copy(out=id_fr[:], in_=id_f32[:])

            def rep_pair(ext_a, ext_b, tag, name):
                t = sb.tile([P, 2, C], f32, tag=tag, name=name)
                nc.sync.dma_start(t[:, 0, :], ext_a[None, :].to_broadcast([P, C]))
                nc.sync.dma_start(t[:, 1, :], ext_b[None, :].to_broadcast([P, C]))
                return t[:, 0, :], t[:, 1, :]

            ln1w_r, ln1b_r = rep_pair(ln1w_ext, ln1b_ext, "repA", "ln1_rep")
            bproj_r, b2_r = rep_pair(bproj_ext, b2_ext, "repB", "res_rep")
            b1_sb = sb.tile([P, FMT], f32)
            nc.sync.dma_start(b1_sb[:], b1_ext.rearrange("(m p) -> p m", p=P))

            # causal masks for diagonal blocks: mask_sh[p, hdup, y] =
            # 1 where key (128*sh + p) <= query y, else 0
            masks = []
            for sh in range(2):
                m = sb.tile([P, 2, CH], bf16, name=f"mask{sh}")
                nc.gpsimd.memset(m[:], 1.0)
                nc.gpsimd.affine_select(
                    out=m[:], in_=m[:], compare_op=ALU.is_ge, fill=0.0,
                    base=-128 * sh, pattern=[[0, 2], [1, CH]],
                    channel_multiplier=-1)
                masks.append(m)
            if debug:
                dbgm = sb.tile([P, 4, CH], f32, tag="T32w", name="dbgm")
                for sh in range(2):
                    nc.vector.tensor_copy(out=dbgm[:, 2 * sh:2 * sh + 2, :],
                                          in_=masks[sh][:])
                nc.sync.dma_start(dbg["mask"], dbgm[:])

            # ---- load x, LN1 -> h (bf16) ----
            x_sb = sb.tile([P, IT, C], f32, tag="T16", name="x_sb")
            for i in range(IT):
                nc.sync.dma_start(x_sb[:, i, :], x_ext[i * P:(i + 1) * P, :])
            h = sb.tile([P, IT, C], bf16, tag="T8h", name="h")
            _layer_norm(nc, sb, st, x_sb, ln1w_r, ln1b_r, h, "ln1")
            for i in range(IT):
                nc.vector.tensor_tensor(out=x_sb[:, i, :], in0=x_sb[:, i, :],
                                        in1=bproj_r[:], op=ALU.add)

            # ---- transpose h -> hT_own [P, KT, T_OWN] bf16 ----
            hT_own = sb.tile([P, KT, T_OWN], bf16, tag="T16b", name="hT_own")
            ag_in = dram.tile([C, T_OWN], bf16)
            ag_out = dram.tile([4 * C, T_OWN], bf16)
            for ct in range(KT):
                for i in range(IT):
                    tp = ps.tile([P, P], bf16, tag="tp", bufs=2,
                                 name=f"tp_h_{i}_{ct}")
                    nc.tensor.transpose(tp[:], h[:, i, ct * P:(ct + 1) * P], id_bf[:])
                    nc.vector.tensor_copy(out=hT_own[:, ct, i * P:(i + 1) * P],
                                          in_=tp[:])
                nc.sync.dma_start(ag_in[ct * P:(ct + 1) * P, :], hT_own[:, ct, :])

            # ---- AllGather hT ----
            nc.gpsimd.collective_compute(
                "AllGather", ALU.bypass, ins=[ag_in.opt()], outs=[ag_out.opt()],
                replica_groups=GROUPS)
            hT_all = sb.tile([P, KT, 4, T_OWN], bf16, tag="T32", name="hT_all")
            for r in range(4):
                nc.sync.dma_start(
                    hT_all[:, :, r, :],
                    ag_out[r * C:(r + 1) * C, :].rearrange(
                        "(kt kp) t -> kp kt t", kp=P))

            if stage < 2:
                return
            # ---- cast W slices to bf16 ----
            wqkv_bf = sb.tile([P, 3, KT, DL], bf16, tag="T16c", name="wqkv_bf")
            for wi, ext in enumerate((wq_ext, wk_ext, wv_ext)):
                wst = sb.tile([P, KT, DL], f32, tag="T32w", bufs=1,
                              name=f"w{wi}_st")
                nc.sync.dma_start(wst[:],
                                  ext.rearrange("(kt kp) d -> kp kt d", kp=P))
                nc.vector.tensor_copy(out=wqkv_bf[:, wi], in_=wst[:])
            wq_bf, wk_bf, wv_bf = wqkv_bf[:, 0], wqkv_bf[:, 1], wqkv_bf[:, 2]
            wp_st = sb.tile([P, 2, C], f32, tag="T32w", bufs=1, name="wp_st")
            nc.sync.dma_start(wp_st[:],
                              wp_ext.rearrange("(kt kp) c -> kp kt c", kp=P))
            wp_bf = sb.tile([P, 2, C], bf16, tag="T4p", name="wp_bf")
            nc.vector.tensor_copy(out=wp_bf[:], in_=wp_st[:])

            # ---- QKV ----
            qT = sb.tile([P, 2, T_ALL], bf16, tag="T8q", name="qT")
            kT_lo = sb.tile([P, 2, T_ALL], bf16, tag="T8k", name="kT_lo")
            kT_hi = sb.tile([P, 2, T_ALL], bf16, tag="T8k2", name="kT_hi")
            nc.vector.memset(kT_lo[64:128, :, :], 0.0)
            nc.vector.memset(kT_hi[0:64, :, :], 0.0)
            v_aug = sb.tile([P, QC * 2, NH, D + 1], bf16, tag="T16b", name="v_aug")
            nc.vector.memset(v_aug[:, :, :, D:D + 1], 1.0)

            for wi, w_bf in enumerate((wq_bf, wk_bf)):
                for mt in range(2):
                    for r in range(4):
                        pp = ps.tile([P, T_OWN], f32, tag="big", bufs=2,
                                     name=f"qkv_{wi}_{mt}_{r}")
                        for kt in range(KT):
                            nc.tensor.matmul(
                                pp[:], w_bf[:, kt, mt * P:(mt + 1) * P],
                                hT_all[:, kt, r, :],
                                start=(kt == 0), stop=(kt == KT - 1))
                        if wi == 0:
                            nc.vector.tensor_copy(
                                out=qT[:, mt, r * T_OWN:(r + 1) * T_OWN], in_=pp[:])
                        else:
                            nc.vector.tensor_copy(
                                out=kT_lo[0:64, mt, r * T_OWN:(r + 1) * T_OWN],
                                in_=pp[0:64, :])
                            nc.vector.tensor_copy(
                                out=kT_hi[64:128, mt, r * T_OWN:(r + 1) * T_OWN],
                                in_=pp[64:128, :])
            for stt in range(QC * 2):
                r, i = stt // IT, stt % IT
                pp = ps.tile([P, T_OWN], f32, tag="big", bufs=2,
                             name=f"v_{stt}")
                for kt in range(KT):
                    nc.tensor.matmul(
                        pp[:, :DL],
                        hT_all[:, kt, r, i * P:(i + 1) * P],
                        wv_bf[:, kt, :],
                        start=(kt == 0), stop=(kt == KT - 1))
                nc.vector.tensor_copy(
                    out=v_aug[:, stt, :, 0:D],
                    in_=pp[:, :DL].rearrange("p (h d) -> p h d", d=D))

            if stage < 3:
                return
            # ---- attention ----
            attn_sb = sb.tile([P, QC * 2, DL], bf16, tag="T8h", name="attn_sb")
            for hp in range(2):
                for qc in range(QC):
                    aps = [ps.tile([P, D + 1], f32, tag="attn", bufs=4,
                                   name=f"attn_{hp}_{qc}_{i}")
                           for i in range(4)]
                    for kc in range(qc + 1):
                        for sh in range(2):
                            sc = ps.tile([P, 2 * CH], f32, tag="big", bufs=2,
                                         name=f"sc_{hp}_{qc}_{kc}_{sh}")
                            for hl in range(2):
                                kTv = kT_lo if hl == 0 else kT_hi
                                nc.tensor.matmul(
                                    sc[:, hl * CH:(hl + 1) * CH],
                                    kTv[:, hp,
                                        kc * CH + sh * P: kc * CH + (sh + 1) * P],
                                    qT[:, hp, qc * CH:(qc + 1) * CH],
                                    start=True, stop=True)
                            ex = st.tile([P, 2 * CH], bf16, tag="expT", bufs=3,
                                         name=f"ex_{hp}_{qc}_{kc}_{sh}")
                            nc.scalar.activation(ex[:], sc[:], ACT_F.Exp,
                                                 bias=0.0, scale=SCALE)
                            if kc == qc:
                                nc.vector.tensor_tensor(
                                    out=ex.rearrange("p (a y) -> p a y", y=CH),
                                    in0=ex.rearrange("p (a y) -> p a y", y=CH),
                                    in1=masks[sh][:], op=ALU.mult)
                            if debug and hp == 0 and qc == 0:
                                dbge = sb.tile([P, 2, 2 * CH], f32, tag="T32w",
                                               name=f"dbge_{sh}")
                                nc.vector.tensor_copy(out=dbge[:, sh, :], in_=ex[:])
                                if sh == 1:
                                    nc.sync.dma_start(dbg["ex"], dbge[:])
                            for hl in range(2):
                                for ti in range(2):
                                    nc.tensor.matmul(
                                        aps[hl * 2 + ti][:],
                                        ex[:, hl * CH + ti * P: hl * CH + (ti + 1) * P],
                                        v_aug[:, 2 * kc + sh, 2 * hp + hl, :],
                                        start=(kc == 0 and sh == 0),
                                        stop=(kc == qc and sh == 1))
                    if debug and hp == 0 and qc == 0:
                        dbga = sb.tile([P, 2 * (D + 1)], f32, name="dbga")
                        nc.vector.tensor_copy(out=dbga[:, 0:D + 1], in_=aps[0][:])
                        nc.vector.tensor_copy(out=dbga[:, D + 1:], in_=aps[1][:])
                        nc.sync.dma_start(dbg["aps"], dbga[:])
                    for hl in range(2):
                        for ti in range(2):
                            a = aps[hl * 2 + ti]
                            rd = st.tile([P, 1], f32, tag="rd", bufs=4,
                                         name=f"rd_{hp}_{qc}_{hl}_{ti}")
                            nc.vector.reciprocal(rd[:], a[:, D:D + 1])
                            nc.vector.tensor_scalar(
                                out=attn_sb[:, 2 * qc + ti,
                                            (2 * hp + hl) * D:(2 * hp + hl + 1) * D],
                                in0=a[:, 0:D],
                                scalar1=rd[:], scalar2=None, op0=ALU.mult)

            if stage < 4:
                return
            # ---- transpose attn -> attnT [P, 2, T_ALL] bf16 ----
            attnT = sb.tile([P, 2, T_ALL], bf16, tag="T8q", name="attnT")
            for tt in range(QC * 2):
                for ct in range(2):
                    tp = ps.tile([P, P], bf16, tag="tp", bufs=2,
                                 name=f"tp_a_{tt}_{ct}")
                    nc.tensor.transpose(tp[:], attn_sb[:, tt, ct * P:(ct + 1) * P],
                                        id_bf[:])
                    nc.vector.tensor_copy(out=attnT[:, ct, tt * P:(tt + 1) * P],
                                          in_=tp[:])

            # ---- proj partial -> rs_dram ----
            rs_in = dram.tile([T_ALL, C], bf16)
            rs_out = dram.tile([T_OWN, C], bf16)
            for mt in range(QC * 2):
                ob = st.tile([P, C], bf16, tag="projev", bufs=2, name=f"projev_{mt}")
                for n in range(2):
                    pp = ps.tile([P, 512], f32, tag="big", bufs=2,
                                 name=f"proj_{mt}_{n}")
                    for kt2 in range(2):
                        nc.tensor.matmul(
                            pp[:], attnT[:, kt2, mt * P:(mt + 1) * P],
                            wp_bf[:, kt2, n * 512:(n + 1) * 512],
                            start=(kt2 == 0), stop=(kt2 == 1))
                    nc.vector.tensor_copy(out=ob[:, n * 512:(n + 1) * 512],
                                          in_=pp[:])
                nc.sync.dma_start(rs_in[mt * P:(mt + 1) * P, :], ob[:])
            nc.gpsimd.collective_compute(
                "ReduceScatter", ALU.add, ins=[rs_in.opt()], outs=[rs_out.opt()],
                replica_groups=GROUPS)

            if stage < 5:
                return
            # ---- residual 1: out1 = x + rs + bproj ----
            rs_sb = sb.tile([P, IT, C], bf16)
            nc.sync.dma_start(rs_sb[:], rs_out.rearrange("(i p) c -> p i c", p=P))
            out1 = sb.tile([P, IT, C], f32, tag="T16c", name="out1")
            for i in range(IT):
                nc.vector.tensor_tensor(out=out1[:, i, :], in0=x_sb[:, i, :],
                                        in1=rs_sb[:, i, :], op=ALU.add)

            # ---- LN2 -> h2 (f32r) ----
            ln2w_r, ln2b_r = rep_pair(ln2w_ext, ln2b_ext, "repA", "ln2_rep")
            h2 = sb.tile([P, IT, C], f32r, tag="T16", name="h2")
            _layer_norm(nc, sb, st, out1, ln2w_r, ln2b_r, h2, "ln2")

            # ---- transpose h2 -> h2T [P, KT, T_OWN] f32r ----
            h2T = sb.tile([P, KT, T_OWN], f32r, tag="T16b", name="h2T")
            for i in range(IT):
                for ct in range(KT):
                    tp = ps.tile([P, P], f32r, tag="tp", bufs=2,
                                 name=f"tp_h2_{i}_{ct}")
                    nc.tensor.transpose(tp[:], h2[:, i, ct * P:(ct + 1) * P],
                                        id_fr[:])
                    nc.vector.tensor_copy(out=h2T[:, ct, i * P:(i + 1) * P],
                                          in_=tp[:])

            if stage < 6:
                return
            # ---- FFN1 (f32r): ff1T[m, t] = relu(W1.T h2T + b1) ----
            ff1T = sb.tile([P, FMT, T_OWN], bf16, tag="T32", name="ff1T")
            for mt in range(FMT):
                w1s = st.tile([P, KT, P], f32r, tag="w1st", bufs=3, name=f"w1st_{mt}")
                nc.sync.dma_start(
                    w1s[:],
                    w1_ext[:, mt * P:(mt + 1) * P].rearrange(
                        "(kt kp) m -> kp kt m", kp=P))
                pp = ps.tile([P, T_OWN], f32, tag="big", bufs=2,
                             name=f"ff1_{mt}")
                for kt in range(KT):
                    nc.tensor.matmul(pp[:], w1s[:, kt, :], h2T[:, kt, :],
                                     start=(kt == 0), stop=(kt == KT - 1))
                nc.scalar.activation(ff1T[:, mt, :], pp[:], ACT_F.Relu,
                                     bias=b1_sb[:, mt:mt + 1])

            # ---- FFN2 (bf16): two n-half passes, W2 streamed+cast per pass ----
            for n in range(2):
                w2h = sb.tile([P, FMT, 512], bf16, tag="T32w", name=f"w2h_{n}")
                for kt in range(FMT):
                    w2s = st.tile([P, 512], f32, tag="w2st", bufs=2,
                                  name=f"w2st_{n}_{kt}")
                    nc.sync.dma_start(
                        w2s[:], w2_ext[kt * P:(kt + 1) * P,
                                       n * 512:(n + 1) * 512])
                    nc.gpsimd.tensor_copy(out=w2h[:, kt, :], in_=w2s[:])
                for m in range(IT):
                    pp = ps.tile([P, 512], f32, tag="big", bufs=2,
                                 name=f"ff2_{m}_{n}")
                    for kt in range(FMT):
                        nc.tensor.matmul(
                            pp[:], ff1T[:, kt, m * P:(m + 1) * P],
                            w2h[:, kt, :],
                            start=(kt == 0), stop=(kt == FMT - 1))
                    ob = st.tile([P, 512], f32, tag="outev", bufs=2,
                                 name=f"outev_{m}_{n}")
                    nc.vector.tensor_tensor(
                        out=ob[:], in0=pp[:],
                        in1=out1[:, m, n * 512:(n + 1) * 512], op=ALU.add)
                    nc.vector.tensor_tensor(
                        out=ob[:], in0=ob[:],
                        in1=b2_r[:, n * 512:(n + 1) * 512], op=ALU.add)
                    nc.sync.dma_start(
                        out_ext[m * P:(m + 1) * P, n * 512:(n + 1) * 512],
                        ob[:])


_NC_CACHE = None


def _get_nc():
    global _NC_CACHE
    if _NC_CACHE is None:
        _NC_CACHE = build()
    return _NC_CACHE


def shard_inputs(x, Wq, Wk, Wv, Wproj, bproj, W1, b1, W2, b2,
                 ln1_w, ln1_b, ln2_w, ln2_b):
    in_maps = []
    for c in range(NCORES):
        b, j = c // 4, c % 4
        hs = slice(DL * j, DL * (j + 1))
        in_maps.append({
            "x": np.ascontiguousarray(x[b, T_OWN * j:T_OWN * (j + 1)], np.float32),
            "wq": np.ascontiguousarray(Wq[:, hs], np.float32),
            "wk": np.ascontiguousarray(Wk[:, hs], np.float32),
            "wv": np.ascontiguousarray(Wv[:, hs], np.float32),
            "wp": np.ascontiguousarray(Wproj[hs, :], np.float32),
            "w1": np.ascontiguousarray(W1, np.float32),
            "w2": np.ascontiguousarray(W2, np.float32),
            "bproj": np.ascontiguousarray(bproj, np.float32),
            "b1": np.ascontiguousarray(b1, np.float32),
            "b2": np.ascontiguousarray(b2, np.float32),
            "ln1w": np.ascontiguousarray(ln1_w, np.float32),
            "ln1b": np.ascontiguousarray(ln1_b, np.float32),
            "ln2w": np.ascontiguousarray(ln2_w, np.float32),
            "ln2b": np.ascontiguousarray(ln2_b, np.float32),
        })
    return in_maps


def assemble(results):
    out = np.empty((2, T_ALL, C), np.float32)
    for c in range(NCORES):
        b, j = c // 4, c % 4
        out[b, T_OWN * j:T_OWN * (j + 1)] = results[c]["out"]
    return out


def kernel(**inputs):
    nc = _get_nc()
    in_maps = shard_inputs(**{k: np.asarray(v) for k, v in inputs.items()})
    res = run_bass_kernel_spmd(nc, in_maps, list(range(NCORES)))
    return assemble(res.results)



# revision 14
# speedup vs baseline: 1.0346x; 1.0346x over previous
"""Transformer block (pre-LN attention + FFN) on 8 TRN2 NeuronCores — v2.

Sharding: cores 0-3 handle batch 0, cores 4-7 batch 1. Core c (b=c//4,
j=c%4) owns heads [4j:4j+4) of batch b for attention, and a strided token
shard for LN2/FFN/residual: rows {512r + 128j + [0,128) : r=0..3}.

Key design points vs v1:
  - LN1 is REPLICATED per batch-group (each core LNs all 2048 tokens of
    its batch) — kills the AllGather entirely.
  - LN gamma/beta are folded into Wq/Wk/Wv/W1 (and ln_b@Wv@Wproj into
    bproj, ln2_b@W1 into b1) on the HOST, so on-chip LN is a pure
    (x-mu)*rstd normalize.
  - QKV and attn*V matmuls run in fp8e4m3 with DoubleRow perf mode (2x);
    weights are pre-scaled by 64 on the host, descaled at PSUM evacuation.
    Scores, proj and FFN stay bf16 (fp8 there breaks the 2e-2 budget).
  - All weights are host-precast (fp8/bf16) and pre-arranged so every DMA
    is contiguous >=2KB per partition row.
  - The proj ReduceScatter is split into 4 token-chunks, each launched as
    soon as attention for that chunk completes, hiding collective latency
    under the remaining attention work.
"""

import numpy as np

import concourse.bass as bass
import concourse.mybir as mybir
import concourse.tile as tile
from concourse import bacc
from concourse.bass_utils import run_bass_kernel_spmd
from concourse.masks import make_identity

P = 128
C = 1024           # n_embd
KT = C // P        # 8 c-tiles
T = 2048           # tokens per batch
IT = T // P        # 16 token tiles
NH = 4             # heads per core
D = 64             # head dim
DL = NH * D        # 256 local head features
FF = 4096
FMT = FF // P      # 32 ffn m-tiles
CH = 256           # attention q-chunk
QC = T // CH       # 8 q-chunks
T_OWN = 512        # own tokens per core (4 strided chunks of 128)
NCHUNK = 4         # RS chunks (512 global tokens each -> 128 own rows)
EPS = 1e-5
SCALE = 1.0 / 32.0   # C ** -0.5 (reference quirk)
WS = 64.0            # fp8 weight prescale
GROUPS = [[0, 1, 2, 3], [4, 5, 6, 7]]
NCORES = 8

f32 = mybir.dt.float32
bf16 = mybir.dt.bfloat16
f8 = mybir.dt.float8e4
AX = mybir.AxisListType
ALU = mybir.AluOpType
ACT_F = mybir.ActivationFunctionType
DR = mybir.MatmulPerfMode.DoubleRow

NP_BF16 = mybir.dt.np(bf16)
NP_F8 = mybir.dt.np(f8)


def build(debug=False):
    nc = bacc.Bacc("TRN2", target_bir_lowering=False, debug=False,
                   num_devices=NCORES)
    _build_graph(nc)
    nc.compile()
    return nc


def _build_graph(nc):
    x_ext = nc.dram_tensor("x", [T, C], bf16, kind="ExternalInput").ap()
    xo_ext = nc.dram_tensor("xo", [T_OWN, C], bf16, kind="ExternalInput").ap()
    wq_ext = nc.dram_tensor("wq", [P, KT * DL], f8, kind="ExternalInput").ap()
    wk_ext = nc.dram_tensor("wk", [P, KT * DL], f8, kind="ExternalInput").ap()
    wv_ext = nc.dram_tensor("wv", [P, KT * DL], f8, kind="ExternalInput").ap()
    wp_ext = nc.dram_tensor("wp", [2, P, C], bf16, kind="ExternalInput").ap()
    w1_ext = nc.dram_tensor("w1", [FMT, P, KT * P], bf16,
                            kind="ExternalInput").ap()
    w2_ext = nc.dram_tensor("w2", [4, P, FMT * 256], bf16,
                            kind="ExternalInput").ap()
    bqk_ext = nc.dram_tensor("bqk", [P, 4], f32, kind="ExternalInput").ap()
    b1_ext = nc.dram_tensor("b1r", [P, FMT], f32, kind="ExternalInput").ap()
    bpb2_ext = nc.dram_tensor("bpb2", [2, C], f32, kind="ExternalInput").ap()
    out_ext = nc.dram_tensor("out", [T_OWN, C], f32, kind="ExternalOutput").ap()

    with tile.TileContext(nc) as tc:
        with (
            tc.tile_pool(name="sb", bufs=1) as sb,
            tc.tile_pool(name="st", bufs=3) as st,
            tc.tile_pool(name="ps", bufs=1, space="PSUM") as ps,
            tc.tile_pool(name="dram", bufs=1, space="DRAM") as dram,
        ):
            # ---- constants ----
            id_f32 = sb.tile([P, P], f32)
            make_identity(nc, id_f32[:])
            id_bf = sb.tile([P, P], bf16)
            nc.vector.tensor_copy(out=id_bf[:], in_=id_f32[:])

            # replicated bias rows: bproj_eff, b2
            bp_b2 = sb.tile([P, 2, C], f32, name="bp_b2")
            for i in range(2):
                nc.sync.dma_start(bp_b2[:, i, :],
                                  bpb2_ext[i][None, :].to_broadcast([P, C]))
            bproj_r, b2_r = bp_b2[:, 0, :], bp_b2[:, 1, :]
            bqk_sb = sb.tile([P, 4], f32, name="bqk")   # [p, (bq0,bq1,bk0,bk1)]
            nc.sync.dma_start(bqk_sb[:], bqk_ext)
            b1_sb = sb.tile([P, FMT], f32, name="b1r")
            nc.sync.dma_start(b1_sb[:], b1_ext)

            # causal masks for diagonal blocks (fp8 0/1), dup for 2 heads:
            # mask_sh[p, hdup, y] = 1 where key (128*sh + p) <= query y
            masks = []
            for sh in range(2):
                m = sb.tile([P, 2, CH], f8, name=f"mask{sh}")
                nc.gpsimd.memset(m[:], 1.0)
                nc.gpsimd.affine_select(
                    out=m[:], in_=m[:], compare_op=ALU.is_ge, fill=0.0,
                    base=-128 * sh, pattern=[[0, 2], [1, CH]],
                    channel_multiplier=-1)
                masks.append(m)

            # ---- QKV weights (fp8, host-prearranged [kp, kt, d]) ----
            wqkv = sb.tile([P, 3, KT, DL], f8, name="wqkv")
            for wi, ext in enumerate((wq_ext, wk_ext, wv_ext)):
                nc.sync.dma_start(
                    wqkv[:, wi], ext.rearrange("p (kt d) -> p kt d", kt=KT))
            wq_sb, wk_sb, wv_sb = wqkv[:, 0], wqkv[:, 1], wqkv[:, 2]
            wp_sb = sb.tile([P, 2, C], bf16, name="wp")
            nc.sync.dma_start(wp_sb[:], wp_ext.rearrange("k p c -> p k c"))

            # ---- load x (bf16, full batch), LN1 stats ----
            x_sb = sb.tile([P, IT, C], bf16, tag="xfull", name="x_sb")
            for i in range(IT):
                nc.sync.dma_start(x_sb[:, i, :], x_ext[i * P:(i + 1) * P, :])
            ssum = sb.tile([P, IT], f32, name="ln1_ssum")
            sqs = sb.tile([P, IT], f32, name="ln1_sqs")
            for i in range(IT):
                nc.vector.tensor_reduce(ssum[:, i:i + 1], x_sb[:, i, :],
                                        AX.X, ALU.add)
                sq = st.tile([P, C], f32, tag="lnsq", bufs=1,
                             name=f"ln1_sq_{i}")
                nc.scalar.activation(sq[:], x_sb[:, i, :], ACT_F.Square,
                                     accum_out=sqs[:, i:i + 1])
            mu = sb.tile([P, IT], f32, name="ln1_mu")
            nc.vector.tensor_scalar_mul(mu[:], ssum[:], 1.0 / C)
            var = sb.tile([P, IT], f32, name="ln1_var")
            nc.vector.tensor_scalar_mul(var[:], sqs[:], 1.0 / C)
            musq = sb.tile([P, IT], f32, name="ln1_musq")
            nc.vector.tensor_mul(out=musq[:], in0=mu[:], in1=mu[:])
            nc.vector.tensor_sub(out=var[:], in0=var[:], in1=musq[:])
            nc.vector.tensor_scalar_add(var[:], var[:], EPS)
            rv = sb.tile([P, IT], f32, name="ln1_rv")
            nc.vector.reciprocal(rv[:], var[:])
            rstd = sb.tile([P, IT], f32, name="ln1_rstd")
            nc.scalar.sqrt(rstd[:], rv[:])

            # ---- normalize in-place -> x_sb becomes h (bf16) ----
            hT = sb.tile([P, KT, T], f8, name="hT")
            qT = sb.tile([P, 2, T], bf16, name="qT")
            kT_lo = sb.tile([P, 2, T], bf16, name="kT_lo")
            kT_hi = sb.tile([P, 2, T], bf16, name="kT_hi")
            nc.vector.memset(kT_lo[64:128, :, :], 0.0)
            nc.vector.memset(kT_hi[0:64, :, :], 0.0)
            v_aug = sb.tile([P, QC * 2, NH, D + 1], f8, name="v_aug")
            nc.vector.memset(v_aug[:, :, :, D:D + 1], 1.0)

            for r in range(4):
                for ii in range(4):
                    i = 4 * r + ii
                    eng = nc.vector if ii % 2 == 0 else nc.gpsimd
                    eng.tensor_scalar(
                        out=x_sb[:, i, :], in0=x_sb[:, i, :],
                        scalar1=mu[:, i:i + 1], scalar2=rstd[:, i:i + 1],
                        op0=ALU.subtract, op1=ALU.mult)
                # transpose h quarter -> hT (fp8 cast at evacuation)
                for ii in range(4):
                    i = 4 * r + ii
                    for ct in range(KT):
                        tp = ps.tile([P, P], bf16, tag="tp", bufs=2,
                                     name=f"tp_h_{i}_{ct}")
                        nc.tensor.transpose(tp[:],
                                            x_sb[:, i, ct * P:(ct + 1) * P],
                                            id_bf[:])
                        if ct % 2 == 0:
                            nc.vector.tensor_copy(
                                out=hT[:, ct, i * P:(i + 1) * P], in_=tp[:])
                        else:
                            nc.scalar.activation(
                                hT[:, ct, i * P:(i + 1) * P], tp[:],
                                ACT_F.Copy)
                # q, k for this quarter (fp8 DoubleRow)
                tsl = slice(r * T_OWN, (r + 1) * T_OWN)
                for wi, w in enumerate((wq_sb, wk_sb)):
                    for mt in range(2):
                        pp = ps.tile([P, T_OWN], f32, tag="big", bufs=2,
                                     name=f"qk_{r}_{wi}_{mt}")
                        for kp in range(KT // 2):
                            nc.tensor.matmul(
                                pp[:], w[:, 2 * kp:2 * kp + 2,
                                         mt * P:(mt + 1) * P],
                                hT[:, 2 * kp:2 * kp + 2, tsl],
                                start=(kp == 0), stop=(kp == KT // 2 - 1),
                                perf_mode=DR)
                        bias = bqk_sb[:, 2 * wi + mt:2 * wi + mt + 1]
                        if wi == 0:
                            nc.vector.tensor_scalar(
                                out=qT[:, mt, tsl], in0=pp[:],
                                scalar1=1.0 / WS, scalar2=bias,
                                op0=ALU.mult, op1=ALU.add)
                        else:
                            nc.vector.tensor_scalar(
                                out=kT_lo[0:64, mt, tsl], in0=pp[0:64, :],
                                scalar1=1.0 / WS, scalar2=bias[0:64],
                                op0=ALU.mult, op1=ALU.add)
                            nc.vector.tensor_scalar(
                                out=kT_hi[64:128, mt, tsl], in0=pp[64:128, :],
                                scalar1=1.0 / WS, scalar2=bias[64:128],
                                op0=ALU.mult, op1=ALU.add)
                # v for this quarter (fp8 DoubleRow, out [tokens, feats])
                for tt in range(4):
                    stt = 4 * r + tt
                    pp = ps.tile([P, DL], f32, tag="big", bufs=2,
                                 name=f"v_{stt}")
                    for kp in range(KT // 2):
                        nc.tensor.matmul(
                            pp[:],
                            hT[:, 2 * kp:2 * kp + 2, stt * P:(stt + 1) * P],
                            wv_sb[:, 2 * kp:2 * kp + 2, :],
                            start=(kp == 0), stop=(kp == KT // 2 - 1),
                            perf_mode=DR)
                    nc.vector.tensor_scalar(
                        out=v_aug[:, stt, :, 0:D],
                        in0=pp[:].rearrange("p (h d) -> p h d", d=D),
                        scalar1=1.0 / WS, scalar2=None, op0=ALU.mult)

            # ---- attention (qc outer for chunked proj/RS) ----
            attn_sb = sb.tile([P, QC * 2, DL], bf16, name="attn_sb")
            attnT = sb.tile([P, 2, T], bf16, name="attnT")
            rs_in = [dram.tile([512, C], bf16, name=f"rs_in_{r}")
                     for r in range(NCHUNK)]
            rs_out = [dram.tile([P, C], bf16, name=f"rs_out_{r}")
                      for r in range(NCHUNK)]
            x_own = sb.tile([P, NCHUNK, C], bf16, name="x_own")
            for r in range(NCHUNK):
                nc.sync.dma_start(x_own[:, r, :], xo_ext[r * P:(r + 1) * P, :])

            for qc in range(QC):
                for hp in range(2):
                    aps = [ps.tile([P, D + 1], f32, tag="attn", bufs=4,
                                   name=f"attn_{hp}_{qc}_{i}")
                           for i in range(4)]
                    for kc in range(qc + 1):
                        ex = st.tile([P, 2, 2 * CH], f8, tag="expT", bufs=3,
                                     name=f"ex_{hp}_{qc}_{kc}")
                        for sh in range(2):
                            sc = ps.tile([P, 2 * CH], f32, tag="big", bufs=2,
                                         name=f"sc_{hp}_{qc}_{kc}_{sh}")
                            for hl in range(2):
                                kTv = kT_lo if hl == 0 else kT_hi
                                nc.tensor.matmul(
                                    sc[:, hl * CH:(hl + 1) * CH],
                                    kTv[:, hp,
                                        kc * CH + sh * P: kc * CH + (sh + 1) * P],
                                    qT[:, hp, qc * CH:(qc + 1) * CH],
                                    start=True, stop=True)
                            nc.scalar.activation(ex[:, sh, :], sc[:], ACT_F.Exp,
                                                 bias=0.0, scale=SCALE)
                            if kc == qc:
                                nc.vector.tensor_tensor(
                                    out=ex[:, sh, :].rearrange(
                                        "p (a y) -> p a y", y=CH),
                                    in0=ex[:, sh, :].rearrange(
                                        "p (a y) -> p a y", y=CH),
                                    in1=masks[sh][:], op=ALU.mult)
                        for hl in range(2):
                            for ti in range(2):
                                nc.tensor.matmul(
                                    aps[hl * 2 + ti][:],
                                    ex[:, :, hl * CH + ti * P:
                                       hl * CH + (ti + 1) * P],
                                    v_aug[:, 2 * kc:2 * kc + 2, 2 * hp + hl, :],
                                    start=(kc == 0), stop=(kc == qc),
                                    perf_mode=DR)
                    for hl in range(2):
                        for ti in range(2):
                            a = aps[hl * 2 + ti]
                            rd = st.tile([P, 1], f32, tag="rd", bufs=4,
                                         name=f"rd_{hp}_{qc}_{hl}_{ti}")
                            nc.vector.reciprocal(rd[:], a[:, D:D + 1])
                            nc.vector.tensor_scalar(
                                out=attn_sb[:, 2 * qc + ti,
                                            (2 * hp + hl) * D:(2 * hp + hl + 1) * D],
                                in0=a[:, 0:D],
                                scalar1=rd[:], scalar2=None, op0=ALU.mult)

                if qc % 2 == 1:
                    r = qc // 2
                    # transpose attn chunk -> attnT, proj, chunked RS
                    for tt in range(4 * r, 4 * r + 4):
                        for ct in range(2):
                            tp = ps.tile([P, P], bf16, tag="tp", bufs=2,
                                         name=f"tp_a_{tt}_{ct}")
                            nc.tensor.transpose(
                                tp[:], attn_sb[:, tt, ct * P:(ct + 1) * P],
                                id_bf[:])
                            nc.scalar.activation(
                                attnT[:, ct, tt * P:(tt + 1) * P], tp[:],
                                ACT_F.Copy)
                    for mt in range(4 * r, 4 * r + 4):
                        ob = st.tile([P, C], bf16, tag="projev", bufs=2,
                                     name=f"projev_{mt}")
                        for n in range(2):
                            pp = ps.tile([P, 512], f32, tag="big", bufs=2,
                                         name=f"proj_{mt}_{n}")
                            for kt2 in range(2):
                                nc.tensor.matmul(
                                    pp[:], attnT[:, kt2, mt * P:(mt + 1) * P],
                                    wp_sb[:, kt2, n * 512:(n + 1) * 512],
                                    start=(kt2 == 0), stop=(kt2 == 1))
                            nc.vector.tensor_copy(
                                out=ob[:, n * 512:(n + 1) * 512], in_=pp[:])
                        nc.sync.dma_start(
                            rs_in[r][(mt - 4 * r) * P:(mt - 4 * r + 1) * P, :],
                            ob[:])
                    nc.gpsimd.collective_compute(
                        "ReduceScatter", ALU.add, ins=[rs_in[r].opt()],
                        outs=[rs_out[r].opt()], replica_groups=GROUPS)


            # ---- per-chunk: residual + LN2 -> h2T ----
            out1 = sb.tile([P, NCHUNK, C], f32, name="out1")
            h2 = sb.tile([P, NCHUNK, C], bf16, tag="hfull", name="h2")
            h2T = sb.tile([P, KT, T_OWN], bf16, name="h2T")
            s2sum = sb.tile([P, NCHUNK], f32, name="ln2_ssum")
            s2qs = sb.tile([P, NCHUNK], f32, name="ln2_sqs")
            mu2 = sb.tile([P, NCHUNK], f32, name="ln2_mu")
            var2 = sb.tile([P, NCHUNK], f32, name="ln2_var")
            rstd2 = sb.tile([P, NCHUNK], f32, name="ln2_rstd")
            for r in range(NCHUNK):
                rs_sb = st.tile([P, C], bf16, tag="rssb", bufs=2,
                                name=f"rs_sb_{r}")
                nc.sync.dma_start(rs_sb[:], rs_out[r][:])
                nc.vector.tensor_tensor(out=out1[:, r, :], in0=x_own[:, r, :],
                                        in1=rs_sb[:], op=ALU.add)
                nc.vector.tensor_tensor(out=out1[:, r, :], in0=out1[:, r, :],
                                        in1=bproj_r[:], op=ALU.add)
                nc.vector.tensor_reduce(s2sum[:, r:r + 1], out1[:, r, :],
                                        AX.X, ALU.add)
                sq = st.tile([P, C], f32, tag="lnsq", bufs=1, name=f"ln2_sq_{r}")
                nc.scalar.activation(sq[:], out1[:, r, :], ACT_F.Square,
                                     accum_out=s2qs[:, r:r + 1])
                nc.vector.tensor_scalar_mul(mu2[:, r:r + 1], s2sum[:, r:r + 1],
                                            1.0 / C)
                nc.vector.tensor_scalar_mul(var2[:, r:r + 1], s2qs[:, r:r + 1],
                                            1.0 / C)
                m2 = st.tile([P, 1], f32, tag="rd", bufs=4, name=f"m2_{r}")
                nc.vector.tensor_mul(out=m2[:], in0=mu2[:, r:r + 1],
                                     in1=mu2[:, r:r + 1])
                nc.vector.tensor_sub(out=var2[:, r:r + 1], in0=var2[:, r:r + 1],
                                     in1=m2[:])
                nc.vector.tensor_scalar_add(var2[:, r:r + 1], var2[:, r:r + 1],
                                            EPS)
                nc.vector.reciprocal(var2[:, r:r + 1], var2[:, r:r + 1])
                nc.scalar.sqrt(rstd2[:, r:r + 1], var2[:, r:r + 1])
                nc.vector.tensor_scalar(
                    out=h2[:, r, :], in0=out1[:, r, :],
                    scalar1=mu2[:, r:r + 1], scalar2=rstd2[:, r:r + 1],
                    op0=ALU.subtract, op1=ALU.mult)
                for ct in range(KT):
                    tp = ps.tile([P, P], bf16, tag="tp", bufs=2,
                                 name=f"tp_h2_{r}_{ct}")
                    nc.tensor.transpose(tp[:], h2[:, r, ct * P:(ct + 1) * P],
                                        id_bf[:])
                    if ct % 2 == 0:
                        nc.vector.tensor_copy(
                            out=h2T[:, ct, r * P:(r + 1) * P], in_=tp[:])
                    else:
                        nc.scalar.activation(
                            h2T[:, ct, r * P:(r + 1) * P], tp[:], ACT_F.Copy)

            # ---- FFN1 (bf16, w1 streamed) ----
            ff1T = sb.tile([P, FMT, T_OWN], bf16, tag="xfull", name="ff1T")
            for mt in range(FMT):
                w1s = st.tile([P, KT, P], bf16, tag="w1st", bufs=3,
                              name=f"w1st_{mt}")
                nc.sync.dma_start(
                    w1s[:], w1_ext[mt].rearrange("p (kt m) -> p kt m", kt=KT))
                pp = ps.tile([P, T_OWN], f32, tag="big", bufs=2,
                             name=f"ff1_{mt}")
                for kt in range(KT):
                    nc.tensor.matmul(pp[:], w1s[:, kt, :], h2T[:, kt, :],
                                     start=(kt == 0), stop=(kt == KT - 1))
                nc.scalar.activation(ff1T[:, mt, :], pp[:], ACT_F.Relu,
                                     bias=b1_sb[:, mt:mt + 1])

            # ---- FFN2 (bf16, w2 streamed in 4 column-quarters) ----
            for n in range(4):
                w2q = st.tile([P, FMT, 256], bf16, tag="w2q", bufs=2,
                              name=f"w2q_{n}")
                nc.sync.dma_start(
                    w2q[:], w2_ext[n].rearrange("p (kt m) -> p kt m", kt=FMT))
                for m in range(NCHUNK):
                    pp = ps.tile([P, 256], f32, tag="big", bufs=2,
                                 name=f"ff2_{m}_{n}")
                    for kt in range(FMT):
                        nc.tensor.matmul(
                            pp[:], ff1T[:, kt, m * P:(m + 1) * P],
                            w2q[:, kt, :],
                            start=(kt == 0), stop=(kt == FMT - 1))
                    ob = st.tile([P, 256], f32, tag="outev", bufs=2,
                                 name=f"outev_{m}_{n}")
                    nc.vector.tensor_tensor(
                        out=ob[:], in0=pp[:],
                        in1=out1[:, m, n * 256:(n + 1) * 256], op=ALU.add)
                    nc.vector.tensor_tensor(
                        out=ob[:], in0=ob[:],
                        in1=b2_r[:, n * 256:(n + 1) * 256], op=ALU.add)
                    nc.sync.dma_start(
                        out_ext[m * P:(m + 1) * P, n * 256:(n + 1) * 256],
                        ob[:])


_NC_CACHE = None


def _get_nc():
    global _NC_CACHE
    if _NC_CACHE is None:
        _NC_CACHE = build()
    return _NC_CACHE


def shard_inputs(x, Wq, Wk, Wv, Wproj, bproj, W1, b1, W2, b2,
                 ln1_w, ln1_b, ln2_w, ln2_b):
    x = np.asarray(x, np.float32)
    f = np.float32
    Wq, Wk, Wv, Wproj = (np.asarray(a, f) for a in (Wq, Wk, Wv, Wproj))
    W1, W2 = np.asarray(W1, f), np.asarray(W2, f)
    bproj, b1, b2 = (np.asarray(a, f) for a in (bproj, b1, b2))
    ln1_w, ln1_b = np.asarray(ln1_w, f), np.asarray(ln1_b, f)
    ln2_w, ln2_b = np.asarray(ln2_w, f), np.asarray(ln2_b, f)

    # fold LN1 gamma into Wq/Wk/Wv; beta contributions:
    #   q/k get ln1_b @ W as a per-feature bias; v's goes through proj
    #   into bproj (added exactly once per token on the owning core).
    Wq_f = ln1_w[:, None] * Wq
    Wk_f = ln1_w[:, None] * Wk
    Wv_f = ln1_w[:, None] * Wv
    bq_full = ln1_b @ Wq          # [C]
    bk_full = ln1_b @ Wk
    bproj_eff = bproj + (ln1_b @ Wv) @ Wproj
    # fold LN2 gamma/beta into W1/b1
    W1_f = ln2_w[:, None] * W1
    b1_eff = b1 + ln2_b @ W1

    # host-side layouts
    def qkv8(Wf, hs):   # [C, 256] -> [kp 128, kt 8, d 256] fp8 prescaled
        w = (Wf[:, hs] * WS).reshape(KT, P, DL).transpose(1, 0, 2)
        return np.ascontiguousarray(w.reshape(P, KT * DL)).astype(NP_F8)

    w1h = W1_f.reshape(KT, P, FMT, P).transpose(2, 1, 0, 3)  # [mt, kp, kt, m]
    w1h = np.ascontiguousarray(w1h.reshape(FMT, P, KT * P)).astype(NP_BF16)
    w2h = W2.reshape(FMT, P, 4, 256).transpose(2, 1, 0, 3)   # [n, kp, kt, ni]
    w2h = np.ascontiguousarray(w2h.reshape(4, P, FMT * 256)).astype(NP_BF16)
    b1r = np.ascontiguousarray(b1_eff.reshape(FMT, P).T)     # [p, mt]

    in_maps = []
    for c in range(NCORES):
        b, j = c // 4, c % 4
        hs = slice(DL * j, DL * (j + 1))
        # strided own-token rows: {512r + 128j + [0,128)}
        own_rows = np.concatenate(
            [np.arange(512 * r + P * j, 512 * r + P * j + P)
             for r in range(NCHUNK)])
        wp = Wproj[hs, :].reshape(2, P, C)                   # [kt2, kp, c]
        bqk = np.stack([bq_full[hs].reshape(2, P)[0],
                        bq_full[hs].reshape(2, P)[1],
                        bk_full[hs].reshape(2, P)[0],
                        bk_full[hs].reshape(2, P)[1]], axis=1)  # [p, 4]
        in_maps.append({
            "x": np.ascontiguousarray(x[b]).astype(NP_BF16),
            "xo": np.ascontiguousarray(x[b][own_rows]).astype(NP_BF16),
            "wq": qkv8(Wq_f, hs),
            "wk": qkv8(Wk_f, hs),
            "wv": qkv8(Wv_f, hs),
            "wp": np.ascontiguousarray(wp).astype(NP_BF16),
            "w1": w1h,
            "w2": w2h,
            "bqk": np.ascontiguousarray(bqk, f),
            "b1r": b1r,
            "bpb2": np.ascontiguousarray(np.stack([bproj_eff, b2]), f),
        })
    return in_maps


def assemble(results):
    out = np.empty((2, T, C), np.float32)
    for c in range(NCORES):
        b, j = c // 4, c % 4
        for r in range(NCHUNK):
            out[b, 512 * r + P * j: 512 * r + P * j + P] = \
                results[c]["out"][r * P:(r + 1) * P]
    return out


def kernel(**inputs):
    nc = _get_nc()
    in_maps = shard_inputs(**{k: np.asarray(v) for k, v in inputs.items()})
    res = run_bass_kernel_spmd(nc, in_maps, list(range(NCORES)))
    return assemble(res.results)
